# revision 35
# baseline (speedup 1.0000x reference)
"""Multi-head causal attention (B=4, T=2048, D=1024, H=16) on 8 TRN2 cores.

Tensor-parallel over heads: core c computes heads {2c, 2c+1}. Redesign vs
the previous version:
  - Score matmuls for h0 (PE rows 0-63) and h1 (rows 64-127) are emitted
    back-to-back into separate PSUM banks so the row-tiled pairs execute
    concurrently (~2x score throughput).
  - V is computed directly token-major (lhsT = x^T token slice), so there
    are no PE transposes and no transpose->copy->memset chain.
  - vaug ones-columns are persistent tiles memset once at startup (the old
    per-batch gpsimd memsets stalled PV ~30us/batch behind the norm chain).
  - Norm chain: DVE copy of the denominator row, reciprocal_approx_fast,
    gpsimd partition_broadcast, DVE multiply into y (bf16).
  - Proj drains on DVE as bf16; output DMA'd bf16 and summed on host.
  - Emission is software-pipelined: section b emits QKV(b) interleaved with
    attention of batch b-1 (scores qc0/1 during QKV, PV/proj later), and
    proj(b-1, qc3) is deferred into section b+1.
"""

import sys

for _p in ("/opt/trn_rl_repo",):
    if _p not in sys.path:
        sys.path.append(_p)

import numpy as np
import ml_dtypes

B, T, D = 4, 2048, 1024
H = 16
HD = D // H
NORM = float(np.sqrt(D))
N_CORES = 8
HEADS_PER_CORE = H // N_CORES          # 2
FPC = HEADS_PER_CORE * HD              # 128 features per core
QC = 512                               # query chunk
NQC = T // QC                          # 4
KB = 128                               # key block
DKC = D // 128                         # 8 contraction chunks over D
NTB = T // 128                         # 16 token blocks

_BF16 = ml_dtypes.bfloat16

_cache = {}

N_WARM = 85


def _build():
    import concourse.bacc as bacc
    import concourse.mybir as mybir
    from concourse.tile import TileContext
    from concourse.alu_op_type import AluOpType

    f32 = mybir.dt.float32
    bf16 = mybir.dt.bfloat16
    EXP = mybir.ActivationFunctionType.Exp

    nc = bacc.Bacc("TRN2", target_bir_lowering=False, debug=False,
                   num_devices=N_CORES)

    xt = nc.dram_tensor("xt", [B, D, T], bf16, kind="ExternalInput").ap()
    w3 = nc.dram_tensor("w3", [D, 3 * FPC], bf16, kind="ExternalInput").ap()
    wp = nc.dram_tensor("wp", [FPC, D], bf16, kind="ExternalInput").ap()
    masks = nc.dram_tensor("masks", [4, KB, QC], bf16, kind="ExternalInput").ap()
    out = nc.dram_tensor("out", [B, D, T], bf16, kind="ExternalOutput").ap()

    with TileContext(nc) as tc:
        with (
            tc.tile_pool(name="const", bufs=1) as cpool,
            tc.tile_pool(name="xp", bufs=12) as xpool,
            tc.tile_pool(name="qk", bufs=2) as qkpool,
            tc.tile_pool(name="pt", bufs=32) as ptpool,
            tc.tile_pool(name="y", bufs=6) as ypool,
            tc.tile_pool(name="sm", bufs=2) as smpool,
            tc.tile_pool(name="bcp", bufs=2) as bcpool,
            tc.tile_pool(name="ot", bufs=4) as otpool,
            tc.tile_pool(name="psA", bufs=2, space="PSUM") as psA,
            tc.tile_pool(name="psY", bufs=2, space="PSUM") as psY,
            tc.tile_pool(name="psO", bufs=2, space="PSUM") as psO,
        ):
            # ---- constants ----
            w3_t = []
            for kc in range(DKC):
                t = cpool.tile([128, 3 * FPC], bf16, tag=f"w3{kc}")
                nc.sync.dma_start(t[:], w3[kc * 128:(kc + 1) * 128, :])
                w3_t.append(t)
            wp_t = cpool.tile([FPC, D], bf16, tag="wp")
            nc.sync.dma_start(wp_t[:], wp[:])
            mask_t = []
            for p in range(4):
                t = cpool.tile([KB, QC], bf16, tag=f"mask{p}")
                nc.sync.dma_start(t[:], masks[p])
                mask_t.append(t)

            # persistent vaug tiles: [v_h0 64 | ones | v_h1 64 | ones],
            # double-buffered across batches; ones written once here.
            vaug = [[], []]
            for g in range(2):
                for tb in range(NTB):
                    va = cpool.tile([128, 2 * HD + 2], bf16, tag=f"va{g}_{tb}")
                    nc.gpsimd.memset(va[:, HD:HD + 1], 1.0)
                    nc.gpsimd.memset(va[:, 2 * HD + 1:2 * HD + 2], 1.0)
                    vaug[g].append(va)

            # PE warmup on memset tiles (no DMA dependency): keeps the HAM
            # clock-gate busy while the initial x DMA lands.
            wt = cpool.tile([128, QC], bf16, tag="warm")
            nc.vector.memset(wt[:], 0.25)
            psw = psO.tile([128, QC], f32, tag="pso")
            for _ in range(N_WARM):
                nc.tensor.matmul(psw[:], lhsT=wt[:, 0:128], rhs=wt[:],
                                 start=True, stop=True)

            # ---- mutable cross-section state ----
            xp_t = {}      # b -> [8 tiles]
            qkp = {}       # b -> (qp, kp)
            pts = {}       # (a, qc, h, kb) -> (pt tile, j0)
            y_tiles = {}   # (a, qc) -> y tile

            def prefetch_x(b):
                ts = []
                for kc in range(DKC):
                    t = xpool.tile([128, T], bf16, tag="xp")
                    nc.sync.dma_start(t[:], xt[b, kc * 128:(kc + 1) * 128, :])
                    ts.append(t)
                xp_t[b] = ts

            # ---- QKV units ----
            # Each chunk c is emitted as two interleaved parts so the short
            # N=128 token-major V matmuls hide their weight loads under the
            # long N=512 Q/K streams:
            #   part 0: Q[kc] + V(tb 4c+0)[kc] + V(tb 4c+1)[kc]  for kc=0..7
            #   part 1: K[kc] + V(tb 4c+2)[kc] + V(tb 4c+3)[kc]
            def emit_qk_part(b, c, part):
                if b not in qkp:
                    qp = qkpool.tile([128, T], bf16, tag="qp")
                    kp = qkpool.tile([128, T], bf16, tag="kp")
                    qkp[b] = (qp, kp)
                dst = qkp[b][part]
                ps = psO.tile([128, QC], f32, tag="pso", name="psqk")
                for kc in range(DKC):
                    nc.tensor.matmul(
                        ps[:],
                        lhsT=w3_t[kc][:, 128 * part:128 * (part + 1)],
                        rhs=xp_t[b][kc][:, QC * c:QC * (c + 1)],
                        start=(kc == 0), stop=(kc == DKC - 1),
                    )
                nc.vector.tensor_copy(dst[:, QC * c:QC * (c + 1)], ps[:])

            def emit_v_part(b, c, part):
                # token-major V for two 128-token blocks (lhsT = x^T slice)
                psv = psO.tile([128, QC], f32, tag="pso", name="psv")
                tbs = (4 * c + 2 * part, 4 * c + 2 * part + 1)
                for j, tb in enumerate(tbs):
                    for kc in range(DKC):
                        nc.tensor.matmul(
                            psv[:, 256 * j:256 * j + 128],
                            lhsT=xp_t[b][kc][:, 128 * tb:128 * (tb + 1)],
                            rhs=w3_t[kc][:, 256:384],
                            start=(kc == 0), stop=(kc == DKC - 1),
                        )
                for j, tb in enumerate(tbs):
                    va = vaug[b % 2][tb]
                    nc.vector.tensor_copy(va[:, 0:HD],
                                          psv[:, 256 * j:256 * j + HD])
                    nc.vector.tensor_copy(va[:, HD + 1:2 * HD + 1],
                                          psv[:, 256 * j + HD:256 * j + 128])

            # ---- attention units ----
            def emit_slot(a, qc, kb):
                # one key block, both heads: two concurrent row-tiled MMs
                # into the two banks of one PSUM pair tile, exp on ACT,
                # triangular-boundary mask on DVE.
                j0 = max(0, KB * (kb - 4 * qc))
                p = kb - 4 * qc
                qp, kp = qkp[a][0], qkp[a][1]
                ps = psA.tile([128, 2 * QC], f32, tag="ps")
                for h in range(2):
                    nc.tensor.matmul(
                        ps[:, QC * h + j0:QC * (h + 1)],
                        lhsT=kp[HD * h:HD * (h + 1), KB * kb:KB * (kb + 1)],
                        rhs=qp[HD * h:HD * (h + 1), QC * qc + j0:QC * (qc + 1)],
                        start=True, stop=True,
                    )
                pt = ptpool.tile([KB, 2 * QC], bf16, tag="pt")
                if p < 0:
                    nc.scalar.activation(pt[:], ps[:], EXP, scale=1.0 / NORM)
                else:
                    for h in range(2):
                        nc.scalar.activation(
                            pt[:, QC * h + j0:QC * (h + 1)],
                            ps[:, QC * h + j0:QC * (h + 1)],
                            EXP, scale=1.0 / NORM)
                    for h in range(2):
                        # only the leading 128 columns of a diagonal block
                        # straddle the causal boundary; the rest is unmasked
                        nc.vector.tensor_tensor(
                            pt[:, QC * h + j0:QC * h + j0 + KB],
                            pt[:, QC * h + j0:QC * h + j0 + KB],
                            mask_t[0][:, 0:KB],
                            op=AluOpType.mult,
                        )
                for h in range(2):
                    pts[a, qc, h, kb] = (pt, QC * h, j0)

            def emit_pv(a, qc, h, kbs, psy, nkb, state={}):
                for kb in kbs:
                    pt, off, j0 = pts.pop((a, qc, h, kb))
                    i = state.get((a, qc, h), 0)
                    nc.tensor.matmul(
                        psy[0:HD + 1, j0:QC],
                        lhsT=vaug[a % 2][kb][:, (HD + 1) * h:(HD + 1) * (h + 1)],
                        rhs=pt[:, off + j0:off + QC],
                        start=(i == 0), stop=(i == nkb - 1),
                    )
                    state[a, qc, h] = i + 1

            def emit_drain(a, qc, h, psy):
                if (a, qc) not in y_tiles:
                    y_tiles[a, qc] = ypool.tile([FPC, QC], bf16, tag="y", name="y")
                y = y_tiles[a, qc]
                srow = smpool.tile([1, QC], f32, tag=f"srow{h}")
                yu = smpool.tile([HD, QC], f32, tag=f"yu{h}")
                if qc == 3:
                    # chunk-3 drains run when ACT has no exp work left
                    nc.scalar.copy(srow[:], psy[HD:HD + 1, :])
                    nc.scalar.copy(yu[:], psy[0:HD, :])
                else:
                    nc.vector.tensor_copy(srow[:], psy[HD:HD + 1, :])
                    nc.vector.tensor_copy(yu[:], psy[0:HD, :])
                rec = smpool.tile([1, QC], f32, tag=f"rec{h}")
                nc.vector.reciprocal_approx_fast(rec[:], srow[:])
                bc = bcpool.tile([HD, QC], f32, tag=f"bc{h}")
                nc.gpsimd.partition_broadcast(bc[:], rec[:])
                nc.vector.tensor_tensor(y[HD * h:HD * (h + 1), :], yu[:],
                                        bc[:], op=AluOpType.mult)

            def emit_proj(a, qc, mts):
                y = y_tiles[a, qc]
                for mt in mts:
                    pso = psO.tile([128, QC], f32, tag="pso")
                    nc.tensor.matmul(
                        pso[:],
                        lhsT=wp_t[:, 128 * mt:128 * (mt + 1)],
                        rhs=y[:],
                        start=True, stop=True,
                    )
                    ot = otpool.tile([128, QC], bf16, tag="ot")
                    if qc == 3:
                        # chunk-3 proj drains overlap the next section's
                        # QKV phase, where ACT is otherwise idle
                        nc.scalar.copy(ot[:], pso[:])
                    else:
                        nc.vector.tensor_copy(ot[:], pso[:])
                    nc.sync.dma_start(
                        out[a, 128 * mt:128 * (mt + 1), QC * qc:QC * (qc + 1)],
                        ot[:],
                    )

            def kb_order(qc):
                nkb = 4 * (qc + 1)
                return ([kb for kb in range(nkb) if kb < 4 * qc] +
                        [kb for kb in range(nkb) if kb >= 4 * qc])

            def attn_units(a):
                # ordered attention stream for batch a; yields callables.
                units = []

                def slot_u(qc, kb):
                    units.append(lambda: emit_slot(a, qc, kb))

                psy_tiles = {}

                def pv_u(qc, h, kbs):
                    def f():
                        if (qc, h) not in psy_tiles:
                            psy_tiles[qc, h] = psY.tile([HD + 1, QC], f32,
                                                        tag="psy", name="psy")
                        emit_pv(a, qc, h, kbs, psy_tiles[qc, h], 4 * (qc + 1))
                    units.append(f)

                def drain_u(qc, h):
                    units.append(lambda: emit_drain(a, qc, h, psy_tiles[qc, h]))

                def proj_u(qc, mts):
                    units.append(lambda: emit_proj(a, qc, list(mts)))

                # scores for chunks 0 and 1 feed ACT early (these land
                # interleaved into QKV(a+1) via merge())
                for kb in range(4):
                    slot_u(0, kb)
                for kb in range(8):
                    slot_u(1, kb)
                ko0, ko1 = kb_order(0), kb_order(1)
                ko2, ko3 = kb_order(2), kb_order(3)
                pv_u(0, 0, ko0[:2]); slot_u(2, 0)
                pv_u(0, 1, ko0[:2]); slot_u(2, 1)
                pv_u(0, 0, ko0[2:]); drain_u(0, 0); slot_u(2, 2)
                pv_u(0, 1, ko0[2:]); drain_u(0, 1); slot_u(2, 3)
                slot_u(2, 4); pv_u(1, 0, ko1[:3])
                slot_u(2, 5); pv_u(1, 1, ko1[:3])
                slot_u(2, 6); pv_u(1, 0, ko1[3:6])
                slot_u(2, 7); pv_u(1, 1, ko1[3:6])
                slot_u(2, 8); pv_u(1, 0, ko1[6:]); drain_u(1, 0)
                slot_u(2, 9); pv_u(1, 1, ko1[6:]); drain_u(1, 1)
                slot_u(2, 10); proj_u(0, [0, 1])
                slot_u(2, 11); proj_u(0, [2, 3])
                slot_u(3, 0); proj_u(0, [4, 5])
                slot_u(3, 1); proj_u(0, [6, 7])
                slot_u(3, 2); pv_u(2, 0, ko2[:3])
                slot_u(3, 3); pv_u(2, 1, ko2[:3])
                slot_u(3, 4); pv_u(2, 0, ko2[3:6])
                slot_u(3, 5); pv_u(2, 1, ko2[3:6])
                slot_u(3, 6); pv_u(2, 0, ko2[6:9])
                slot_u(3, 7); pv_u(2, 1, ko2[6:9])
                slot_u(3, 8); pv_u(2, 0, ko2[9:]); drain_u(2, 0)
                slot_u(3, 9); pv_u(2, 1, ko2[9:]); drain_u(2, 1)
                slot_u(3, 10); proj_u(1, [0, 1])
                slot_u(3, 11); proj_u(1, [2, 3])
                slot_u(3, 12); proj_u(1, [4, 5])
                slot_u(3, 13); proj_u(1, [6, 7])
                slot_u(3, 14); pv_u(3, 0, ko3[:3])
                slot_u(3, 15); pv_u(3, 1, ko3[:3])
                pv_u(3, 0, ko3[3:6]); proj_u(2, [0, 1])
                pv_u(3, 1, ko3[3:6]); proj_u(2, [2, 3])
                pv_u(3, 0, ko3[6:9]); proj_u(2, [4, 5])
                pv_u(3, 1, ko3[6:9]); proj_u(2, [6, 7])
                pv_u(3, 0, ko3[9:12])
                pv_u(3, 1, ko3[9:12])
                pv_u(3, 0, ko3[12:]); drain_u(3, 0)
                pv_u(3, 1, ko3[12:]); drain_u(3, 1)
                # proj(3) deferred to the next section
                # first 12 units are the chunk-0/1 score slots: emitted at
                # the END of section a (right after QKV(a)), the rest in
                # section a+1
                return units[:12], units[12:]

            def qkv_units(b):
                units = []
                for c in range(NQC):
                    units.append(lambda c=c: emit_qk_part(b, c, 0))
                    units.append(lambda c=c: emit_v_part(b, c, 0))
                    units.append(lambda c=c: emit_qk_part(b, c, 1))
                    units.append(lambda c=c: emit_v_part(b, c, 1))
                return units

            def merge(fill, attn):
                # interleave: lead with 2 fillers, then spread the rest
                # evenly through the attention stream.
                seq = []
                lead = fill[:2]
                rest = fill[2:]
                seq += lead
                if not attn:
                    return seq + rest
                if not rest:
                    return seq + attn
                stride = max(1, len(attn) // len(rest))
                ai = 0
                for i, f in enumerate(rest):
                    nxt = min(len(attn), (i + 1) * stride)
                    seq += attn[ai:nxt]
                    seq.append(f)
                    ai = nxt
                seq += attn[ai:]
                return seq

            # ---- sections ----
            prefetch_x(0)
            prefetch_x(1)
            pending_rest = []
            for b in range(B + 1):
                fill = []
                if b < B:
                    fill += qkv_units(b)
                if b >= 2:
                    a2 = b - 2
                    fill.append(lambda a2=a2: emit_proj(a2, 3, [0, 1, 2, 3]))
                    fill.append(lambda a2=a2: emit_proj(a2, 3, [4, 5, 6, 7]))
                for u in merge(fill, pending_rest):
                    u()
                pending_rest = []
                if b < B:
                    p0, pending_rest = attn_units(b)
                    for u in p0:
                        u()
                if b + 2 <= B - 1:
                    prefetch_x(b + 2)
            # tail: proj(B-1, 3)
            emit_proj(B - 1, 3, list(range(8)))

    nc.compile()
    return nc


def _get_nc():
    if "nc" not in _cache:
        _cache["nc"] = _build()
    return _cache["nc"]


def _make_masks():
    i = np.arange(KB)[:, None]
    j = np.arange(QC)[None, :]
    m = np.zeros((4, KB, QC), dtype=np.float32)
    for p in range(4):
        m[p] = (j >= (KB * p + i)).astype(np.float32)
    return m.astype(_BF16)


def shard_inputs(x, w_qkv, w_proj):
    xt = np.ascontiguousarray(np.asarray(x, dtype=np.float32).transpose(0, 2, 1))
    xt = xt.astype(_BF16)
    w_qkv = np.asarray(w_qkv, dtype=np.float32)
    w_proj = np.asarray(w_proj, dtype=np.float32)
    masks = _make_masks()
    in_maps = []
    for c in range(N_CORES):
        qcols = slice(FPC * c, FPC * (c + 1))
        kcols = slice(D + FPC * c, D + FPC * (c + 1))
        vcols = slice(2 * D + FPC * c, 2 * D + FPC * (c + 1))
        w3_c = np.concatenate(
            [w_qkv[:, qcols], w_qkv[:, kcols], w_qkv[:, vcols]], axis=1)
        in_maps.append({
            "xt": xt,
            "w3": np.ascontiguousarray(w3_c).astype(_BF16),
            "wp": np.ascontiguousarray(w_proj[FPC * c:FPC * (c + 1), :]).astype(_BF16),
            "masks": masks,
        })
    return in_maps


def unshard(results):
    total = results[0]["out"].astype(np.float32)
    for r in results[1:]:
        total += r["out"].astype(np.float32)
    return np.ascontiguousarray(total.transpose(0, 2, 1))


def run(inputs, trace=False, **kw):
    from concourse.bass_utils import run_bass_kernel_spmd

    nc = _get_nc()
    in_maps = shard_inputs(inputs["x"], inputs["w_qkv"], inputs["w_proj"])
    res = run_bass_kernel_spmd(nc, in_maps, core_ids=list(range(N_CORES)),
                               trace=trace, **kw)
    return unshard(res.results), res


def kernel(**inputs):
    out, _ = run(inputs, trace=False)
    return out
